# revision 1
# baseline (speedup 1.0000x reference)
"""Multi-head attention (B=2, S=2048, D=1024, H=16 heads, causal) on 8 trn2 cores.

Sharding: heads across cores (2 heads = 128 channels per core).
  - W_q/W_k/W_v column-sharded: each core projects all tokens to its 128 channels.
  - Attention per (batch, head) fully local to a core.
  - W_o row-sharded: each core computes a partial output projection; partials
    are summed on the host (the unshard step), then b_o is added.

Device layout: everything transposed (channels on partitions, tokens on free).
  - Scores computed as S^T blocks [128 k-tok, 512 q-tok] so exp is elementwise
    and the softmax sum comes for free from a ones-column appended to V.
  - Causal structure: host inspects the mask and emits only non-empty blocks;
    all-invalid q-ranges are zeroed post-exp, mixed ranges multiplied by 0/1
    pattern tiles (deduplicated; one 128-wide triangle tile for causal).
  - Projections, V-transposes and attention interleave at 512-token
    granularity so the causal columns start as soon as their K/V prefix is
    projected.

All matmuls run in bf16 (inputs cast on host) with fp32 PSUM accumulation;
the partial output is returned bf16 and reduced in fp32 on the host.
"""

import sys

import numpy as np

try:
    import concourse.bass as bass  # noqa: F401
except ImportError:  # pragma: no cover
    sys.path.insert(0, "/opt/trn_rl_repo")

import ml_dtypes

import concourse.mybir as mybir
import concourse.tile as tile
from concourse import bacc, bass_utils
from concourse.masks import make_identity

P = 128
B, S, D = 2, 2048, 1024
H, DK = 16, 64
N_CORES = 8
HPC = H // N_CORES  # heads per core = 2
CH = HPC * DK  # channels per core = 128
TOK = B * S  # 4096
NKB = S // P  # k-blocks per batch = 16
CW = 512  # q column width
NJ = S // CW  # q columns per batch = 4
NTG = S // CW  # 512-token projection groups per batch = 4
KPG = CW // P  # k-blocks per token group = 4
XC = D // P  # x-dim chunks = 8
MO = D // P  # output-channel chunks = 8

BF16 = mybir.dt.bfloat16
F32 = mybir.dt.float32
NPBF16 = ml_dtypes.bfloat16

_BUILD_CACHE = {}


def _runs(flags):
    """Maximal runs of True in a 1-D bool array, as (start, end) pairs."""
    out = []
    i = 0
    n = len(flags)
    while i < n:
        if flags[i]:
            j = i
            while j < n and flags[j]:
                j += 1
            out.append((i, j))
            i = j
        else:
            i += 1
    return tuple(out)


def _analyze_mask(mask):
    """Block plan from the (1,1,S,S) boolean mask (shared across batch/head).

    plan[j] = tuple of (bk, zeros, mixed) for each k-block with any valid
    entry; zeros = (q0,q1) ranges fully invalid (memset post-exp), mixed =
    (pat_off, q0, width) range needing a 0/1 multiply. Patterns are
    deduplicated and concatenated into pats (P, W_total) in [k, q] layout.
    """
    m = np.asarray(mask).reshape(S, S).astype(bool)  # m[q, k]
    pat_index = {}
    pat_list = []
    plan = []
    for j in range(NJ):
        q0 = j * CW
        blocks = []
        for bk in range(NKB):
            sub = m[q0 : q0 + CW, bk * P : (bk + 1) * P]  # (CW q, P k)
            valid_all = sub.all(axis=1)
            if not sub.any():
                continue
            mixed = None
            if not valid_all.all():
                # hull of all not-fully-valid q columns -> one multiply
                idx = np.where(~valid_all)[0]
                a, b_ = int(idx[0]), int(idx[-1]) + 1
                patt = np.ascontiguousarray(sub[a:b_, :].T).astype(np.float32)
                key = (patt.shape[1], patt.tobytes())
                if key not in pat_index:
                    pat_index[key] = len(pat_list)
                    pat_list.append(patt)
                mixed = (pat_index[key], a, b_ - a)
            blocks.append((bk, mixed))
        plan.append(tuple(blocks))
    offs = [0]
    for p_ in pat_list:
        offs.append(offs[-1] + p_.shape[1])
    # bake pattern offsets into the plan
    plan2 = []
    for col in plan:
        col2 = []
        for bk, mixed in col:
            if mixed is not None:
                pid, a, w = mixed
                mixed = (offs[pid], a, w)
            col2.append((bk, mixed))
        plan2.append(tuple(col2))
    if pat_list:
        pat_arr = np.concatenate(pat_list, axis=1)  # (P, W_total)
    else:
        pat_arr = np.ones((P, 1), np.float32)
    return tuple(plan2), pat_arr


def _build(plan, pat_w):
    nc = bacc.Bacc(
        "TRN2",
        target_bir_lowering=False,
        debug=False,
        enable_asserts=True,
        num_devices=N_CORES,
    )
    NTT = B * NTG
    xq = nc.dram_tensor("xq", [NTT, P, XC, CW], BF16, kind="ExternalInput").ap()
    xk = nc.dram_tensor("xk", [NTT, P, XC, CW], BF16, kind="ExternalInput").ap()
    xv = nc.dram_tensor("xv", [NTT, P, XC, CW], BF16, kind="ExternalInput").ap()
    wq = nc.dram_tensor("wq", [D, CH], BF16, kind="ExternalInput").ap()
    wk = nc.dram_tensor("wk", [D, CH], BF16, kind="ExternalInput").ap()
    wv = nc.dram_tensor("wv", [D, CH], BF16, kind="ExternalInput").ap()
    wo = nc.dram_tensor("wo", [CH, D], BF16, kind="ExternalInput").ap()
    bq = nc.dram_tensor("bq", [CH, 1], F32, kind="ExternalInput").ap()
    bk_ = nc.dram_tensor("bk", [CH, 1], F32, kind="ExternalInput").ap()
    bv = nc.dram_tensor("bv", [CH, 1], F32, kind="ExternalInput").ap()
    mpat = nc.dram_tensor("mpat", [P, pat_w], BF16, kind="ExternalInput").ap()
    out = nc.dram_tensor(
        "out", [MO, B * NJ, P, CW], BF16, kind="ExternalOutput"
    ).ap()

    # which token group each attention column must wait for
    attn_after = [max((bk for bk, _ in col), default=0) // KPG for col in plan]

    with tile.TileContext(nc) as tc:
        with (
            tc.tile_pool(name="const", bufs=1) as const,
            tc.tile_pool(name="persist", bufs=1) as persist,
            tc.tile_pool(name="xt", bufs=6) as xtp,
            tc.tile_pool(name="a2", bufs=3) as a2p,
            tc.tile_pool(name="u128", bufs=3) as up,
            tc.tile_pool(name="yt", bufs=4) as ytp,
            tc.tile_pool(name="ob", bufs=3) as obp,
            tc.tile_pool(name="small", bufs=3) as small,
            tc.tile_pool(name="dram", bufs=3, space="DRAM") as dramp,
            tc.tile_pool(name="pp", bufs=2, space="PSUM") as pp,
            tc.tile_pool(name="s2", bufs=2, space="PSUM") as s2p,
            tc.tile_pool(name="op", bufs=2, space="PSUM") as opp,
        ):
            ident = const.tile([P, P], BF16, tag="ident")
            make_identity(nc, ident)

            w_sb = {}
            b_sb = {}
            for name, wdram, bdram in (
                ("q", wq, bq),
                ("k", wk, bk_),
                ("v", wv, bv),
            ):
                w_sb[name] = const.tile(
                    [P, XC, CH], BF16, tag=f"w{name}", name=f"w{name}"
                )
                nc.sync.dma_start(
                    w_sb[name][:], wdram.rearrange("(o p) c -> p o c", p=P)
                )
                b_sb[name] = const.tile([CH, 1], F32, tag=f"b{name}", name=f"b{name}")
                nc.sync.dma_start(b_sb[name][:], bdram)
            wo_sb = const.tile([CH, D], BF16, tag="wo")
            mask_sb = const.tile([P, pat_w], BF16, tag="mpat")

            # V with a trailing ones column, per (batch, local head): [k, d+1]
            vaug = {}
            for b in range(B):
                for hl in range(HPC):
                    t = persist.tile(
                        [P, NKB, DK + 1],
                        BF16,
                        tag=f"vaug{b}{hl}",
                        name=f"vaug{b}{hl}",
                    )
                    nc.gpsimd.memset(t[:, :, DK : DK + 1], 1.0)
                    vaug[b, hl] = t

            qt, kt, vt = {}, {}, {}
            for b in range(B):
                for name, dst in (("k", kt), ("q", qt), ("v", vt)):
                    dst[b] = persist.tile(
                        [CH, S], BF16, tag=f"{name}t{b}", name=f"{name}t{b}"
                    )

            def project(b, name, xdram, tg):
                """One 512-token group of the q/k/v projection for batch b."""
                dst = {"q": qt, "k": kt, "v": vt}[name]
                g = b * NTG + tg
                xt = xtp.tile([P, XC, CW], BF16, tag="xt")
                for h in range(0, XC, 4):
                    nc.sync.dma_start(
                        xt[:, h : h + 4, :], xdram[g, :, h : h + 4, :]
                    )
                ps = pp.tile([CH, CW], F32, tag="pp")
                for xc in range(XC):
                    nc.tensor.matmul(
                        ps[:],
                        lhsT=w_sb[name][:, xc, :],
                        rhs=xt[:, xc, :],
                        start=(xc == 0),
                        stop=(xc == XC - 1),
                    )
                nc.vector.tensor_add(
                    dst[b][:, tg * CW : (tg + 1) * CW],
                    ps[:],
                    b_sb[name][:, 0:1].to_broadcast((CH, CW)),
                )

            def oproj_col(tcol, yt):
                for mo in range(MO):
                    op_ps = pp.tile([P, CW], F32, tag="pp")
                    nc.tensor.matmul(
                        op_ps[:],
                        lhsT=wo_sb[:, mo * P : (mo + 1) * P],
                        rhs=yt[:],
                        start=True,
                        stop=True,
                    )
                    ob = obp.tile([P, CW], BF16, tag="ob")
                    if mo % 2 == 0:
                        nc.scalar.copy(ob[:], op_ps[:])
                    else:
                        nc.vector.tensor_copy(ob[:], op_ps[:])
                    nc.sync.dma_start(out[mo, tcol], ob[:])

            def attention_col(b, j):
                blocks = plan[j]
                q0 = j * CW
                yt = ytp.tile([CH, CW], BF16, tag="yt")
                if not blocks:
                    nc.gpsimd.memset(yt[:], 0.0)
                else:
                    ops = {}
                    for hl in range(HPC):
                        ops[hl] = opp.tile([DK + 1, CW], F32, tag="op", name=f"op{hl}")
                    nblk = len(blocks)

                    def emit_av(i, bk, a2):
                        for hl in range(HPC):
                            nc.tensor.matmul(
                                ops[hl][:],
                                lhsT=vaug[b, hl][:, bk, :],
                                rhs=a2[:, hl, :],
                                start=(i == 0),
                                stop=(i == nblk - 1),
                            )

                    # software pipeline: AV lags one block behind S/exp so the
                    # exp latency hides behind the next block's S matmuls
                    pend_av = None
                    for i, (bk, mixed) in enumerate(blocks):
                        k0 = bk * P
                        s2 = s2p.tile([P, HPC, CW], F32, tag="s2")
                        for hl in range(HPC):
                            hs = slice(hl * DK, (hl + 1) * DK)
                            nc.tensor.matmul(
                                s2[:, hl, :],
                                lhsT=kt[b][hs, k0 : k0 + P],
                                rhs=qt[b][hs, q0 : q0 + CW],
                                start=True,
                                stop=True,
                            )
                        a2 = a2p.tile([P, HPC, CW], BF16, tag="a2")
                        nc.scalar.activation(
                            a2[:],
                            s2[:],
                            mybir.ActivationFunctionType.Exp,
                            scale=0.125,
                        )
                        if mixed is not None:
                            off, a_, w_ = mixed
                            nc.vector.tensor_tensor(
                                a2[:, :, a_ : a_ + w_],
                                a2[:, :, a_ : a_ + w_],
                                mask_sb[:, None, off : off + w_].to_broadcast(
                                    (P, HPC, w_)
                                ),
                                mybir.AluOpType.mult,
                            )
                        if pend_av is not None:
                            emit_av(*pend_av)
                        pend_av = (i, bk, a2)
                    emit_av(*pend_av)
                    # reciprocal of the sums row, broadcast via DRAM bounce,
                    # then normalize straight out of PSUM into yt
                    drt = dramp.tile([HPC, CW], F32, tag="drt")
                    for hl in range(HPC):
                        sums1 = small.tile([1, CW], F32, tag="sums1", name=f"sums{hl}")
                        nc.vector.tensor_copy(sums1[:], ops[hl][DK : DK + 1, :])
                        rec1 = small.tile([1, CW], F32, tag="rec1", name=f"rec{hl}")
                        nc.vector.reciprocal_approx_fast(out=rec1[:], in_=sums1[:])
                        nc.sync.dma_start(drt[hl : hl + 1, :], rec1[:])
                    scale = small.tile([CH, CW], F32, tag="scale")
                    nc.sync.dma_start(
                        scale[:],
                        drt[:, None, :].to_broadcast((HPC, DK, CW)),
                    )
                    for hl in range(HPC):
                        nc.vector.tensor_tensor(
                            yt[hl * DK : (hl + 1) * DK, :],
                            ops[hl][0:DK, :],
                            scale[hl * DK : (hl + 1) * DK, :],
                            mybir.AluOpType.mult,
                        )
                return yt

            pending = []
            for b in range(B):
                for tg in range(NTG):
                    project(b, "k", xk, tg)
                    project(b, "q", xq, tg)
                    project(b, "v", xv, tg)
                    if b == 0 and tg == 0:
                        nc.sync.dma_start(mask_sb[:], mpat)
                        nc.sync.dma_start(wo_sb[:], wo)
                    for tcol, yt in pending:
                        oproj_col(tcol, yt)
                    pending = []
                    for kb in range(tg * KPG, (tg + 1) * KPG):
                        tp = pp.tile([P, P], BF16, tag="pp")
                        nc.tensor.transpose(
                            tp[:], vt[b][:, kb * P : (kb + 1) * P], ident[:]
                        )
                        for hl in range(HPC):
                            nc.vector.tensor_copy(
                                vaug[b, hl][:, kb, 0:DK],
                                tp[:, hl * DK : (hl + 1) * DK],
                            )
                    for j in range(NJ):
                        if attn_after[j] == tg:
                            yt = attention_col(b, j)
                            pending.append((b * NJ + j, yt))
            for tcol, yt in pending:
                oproj_col(tcol, yt)
    nc.compile()
    return nc


def _get_module(plan, pat_w):
    key = (plan, pat_w)
    if key not in _BUILD_CACHE:
        _BUILD_CACHE[key] = _build(plan, pat_w)
    return _BUILD_CACHE[key]


def _prep_inputs(query, key, value, mask, W_q, b_q, W_k, b_k, W_v, b_v, W_o, b_o):
    def xt_of(x):
        x2 = np.asarray(x, np.float32).reshape(TOK, D)
        xt = x2.T.astype(NPBF16)  # (D, TOK)
        xt = xt.reshape(XC, P, B * NTG, CW).transpose(2, 1, 0, 3)
        return np.ascontiguousarray(xt)  # (NTT, P, XC, CW)

    xq, xk, xv = xt_of(query), xt_of(key), xt_of(value)
    plan, pat_arr = _analyze_mask(mask)
    mpat = np.ascontiguousarray(pat_arr).astype(NPBF16)

    W_q = np.asarray(W_q, np.float32)
    W_k = np.asarray(W_k, np.float32)
    W_v = np.asarray(W_v, np.float32)
    W_o = np.asarray(W_o, np.float32)

    in_maps = []
    for c in range(N_CORES):
        cs = slice(c * CH, (c + 1) * CH)
        in_maps.append(
            {
                "xq": xq,
                "xk": xk,
                "xv": xv,
                "wq": np.ascontiguousarray(W_q[cs, :].T).astype(NPBF16),
                "wk": np.ascontiguousarray(W_k[cs, :].T).astype(NPBF16),
                "wv": np.ascontiguousarray(W_v[cs, :].T).astype(NPBF16),
                "wo": np.ascontiguousarray(W_o[:, cs].T).astype(NPBF16),
                "bq": np.asarray(b_q, np.float32)[cs].reshape(CH, 1).copy(),
                "bk": np.asarray(b_k, np.float32)[cs].reshape(CH, 1).copy(),
                "bv": np.asarray(b_v, np.float32)[cs].reshape(CH, 1).copy(),
                "mpat": mpat,
            }
        )
    return plan, mpat.shape[1], in_maps


def run(inputs, trace=False, trace_cores=None):
    """Build (cached), run on 8 cores, return (final_output, BassKernelResults)."""
    plan, pat_w, in_maps = _prep_inputs(**inputs)
    nc = _get_module(plan, pat_w)
    res = bass_utils.run_bass_kernel_spmd(
        nc,
        in_maps,
        core_ids=list(range(N_CORES)),
        trace=trace,
        trace_cores=trace_cores,
    )
    acc = np.zeros((MO, B * NJ, P, CW), np.float32)
    for c in range(N_CORES):
        acc += res.results[c]["out"].astype(np.float32)
    acc = acc.transpose(0, 2, 1, 3).reshape(D, TOK)
    final = acc.T + np.asarray(inputs["b_o"], np.float32)[None, :]
    return final.reshape(B, S, D), res


def kernel(**inputs):
    return run(inputs, trace=False)[0]



# revision 8
# speedup vs baseline: 1.2987x; 1.2987x over previous
"""Multi-head attention (B=2, S=2048, D=1024, H=16 heads, causal) on 8 trn2 cores.

Sharding: heads across cores (2 heads = 128 channels per core).
  - W_q/W_k/W_v column-sharded: each core projects all tokens to its 128 channels.
  - Attention per (batch, head) fully local to a core.
  - W_o row-sharded: each core computes a partial output projection; partials
    are summed on the host (the unshard step), then b_o (+ W_o @ b_v) is added.

Device layout: Q/K transposed (channels on partitions, tokens on free).
  - Scores computed as S^T blocks [128 k-tok, 512 q-tok] so exp is elementwise.
  - V^T produced directly by the projection (lhsT=x chunk, rhs=W_v chunk), no
    PE transposes. Each (b, head) V block carries 64 all-ones columns, so the
    AV matmul emits the softmax sums pre-broadcast across 64 partitions; the
    normalize is then a single tensor-tensor divide per head.
  - Causal structure: scores/exp/AV restricted to the valid q-range per
    k-block; the 127-wide diagonal triangle is multiplied in on GpSimd.
  - Projections / V^T pieces / output-projection pieces are emitted as filler
    between attention blocks so the PE never idles (keeps the 2.4 GHz p-state)
    while the scalar engine runs the exp stream.

All matmuls run in bf16 (inputs cast on host) with fp32 PSUM accumulation;
the partial output is returned bf16 and reduced in fp32 on the host.
"""

import sys
from collections import deque
from functools import partial

import numpy as np

try:
    import concourse.bass as bass  # noqa: F401
except ImportError:  # pragma: no cover
    sys.path.insert(0, "/opt/trn_rl_repo")

import ml_dtypes

import concourse.mybir as mybir
import concourse.tile as tile
from concourse import bacc, bass_utils

P = 128
B, S, D = 2, 2048, 1024
H, DK = 16, 64
N_CORES = 8
HPC = H // N_CORES  # heads per core = 2
CH = HPC * DK  # channels per core = 128
TOK = B * S  # 4096
NKB = S // P  # k-blocks per batch = 16
CW = 512  # q column width
NJ = S // CW  # q columns per batch = 4
NTG = S // CW  # 512-token projection groups per batch = 4
KPG = CW // P  # k-blocks per token group = 4
XC = D // P  # x-dim chunks = 8
MO = D // P  # output-channel chunks = 8

BF16 = mybir.dt.bfloat16
F32 = mybir.dt.float32
NPBF16 = ml_dtypes.bfloat16

_BUILD_CACHE = {}


def _analyze_mask(mask):
    """Block plan from the (1,1,S,S) boolean mask (shared across batch/head).

    plan[j] = tuple of (bk, qa, mixed) for each k-block with any valid entry:
      qa    = first local q with any valid k; scores/exp/AV cover [qa, CW).
      mixed = None or (pat_off, a, w): a2[:, :, a:a+w] *= pattern columns.
    Patterns are deduplicated and concatenated into pats (P, W_total) in
    [k, q] layout.
    """
    m = np.asarray(mask).reshape(S, S).astype(bool)  # m[q, k]
    pat_index = {}
    pat_list = []
    plan = []
    for j in range(NJ):
        q0 = j * CW
        blocks = []
        first = True
        for bk in range(NKB):
            sub = m[q0 : q0 + CW, bk * P : (bk + 1) * P]  # (CW q, P k)
            anyv = sub.any(axis=1)
            if not anyv.any():
                continue
            qa = int(np.argmax(anyv))
            if first:
                # the first block initializes the whole PSUM accumulator
                qa = 0
                first = False
            validall = sub.all(axis=1)
            nfv = ~validall
            nfv[:qa] = False
            mixed = None
            if nfv.any():
                idx = np.where(nfv)[0]
                a_, b_ = int(idx[0]), int(idx[-1]) + 1
                patt = np.ascontiguousarray(sub[a_:b_, :].T).astype(np.float32)
                key = (patt.shape[1], patt.tobytes())
                if key not in pat_index:
                    pat_index[key] = len(pat_list)
                    pat_list.append(patt)
                mixed = (pat_index[key], a_, b_ - a_)
            blocks.append((bk, qa, mixed))
        plan.append(tuple(blocks))
    offs = [0]
    for p_ in pat_list:
        offs.append(offs[-1] + p_.shape[1])
    plan2 = []
    for col in plan:
        col2 = []
        for bk, qa, mixed in col:
            if mixed is not None:
                pid, a_, w_ = mixed
                mixed = (offs[pid], a_, w_)
            col2.append((bk, qa, mixed))
        plan2.append(tuple(col2))
    if pat_list:
        pat_arr = np.concatenate(pat_list, axis=1)  # (P, W_total)
    else:
        pat_arr = np.ones((P, 1), np.float32)
    return tuple(plan2), pat_arr


def _build(plan, pat_w):
    nc = bacc.Bacc(
        "TRN2",
        target_bir_lowering=False,
        debug=False,
        enable_asserts=True,
        num_devices=N_CORES,
    )
    NTT = B * NTG
    xq = nc.dram_tensor("xq", [NTT, P, XC, CW], BF16, kind="ExternalInput").ap()
    xk = nc.dram_tensor("xk", [NTT, P, XC, CW], BF16, kind="ExternalInput").ap()
    xv = nc.dram_tensor("xv", [NTT, P, XC, CW], BF16, kind="ExternalInput").ap()
    wq = nc.dram_tensor("wq", [D, CH], BF16, kind="ExternalInput").ap()
    wk = nc.dram_tensor("wk", [D, CH], BF16, kind="ExternalInput").ap()
    wv = nc.dram_tensor("wv", [D, CH], BF16, kind="ExternalInput").ap()
    wo = nc.dram_tensor("wo", [CH, D], BF16, kind="ExternalInput").ap()
    bq = nc.dram_tensor("bq", [CH, 1], F32, kind="ExternalInput").ap()
    bk_ = nc.dram_tensor("bk", [CH, 1], F32, kind="ExternalInput").ap()
    mpat = nc.dram_tensor("mpat", [P, pat_w], BF16, kind="ExternalInput").ap()
    out = nc.dram_tensor(
        "out", [MO, B * NJ, P, CW], BF16, kind="ExternalOutput"
    ).ap()

    xdram = {"q": xq, "k": xk, "v": xv}

    with tile.TileContext(nc) as tc:
        with (
            tc.tile_pool(name="const", bufs=1) as const,
            tc.tile_pool(name="persist", bufs=1) as persist,
            tc.tile_pool(name="xt", bufs=2) as xtp,
            tc.tile_pool(name="a2", bufs=3) as a2p,
            tc.tile_pool(name="nrm", bufs=2) as nrm,
            tc.tile_pool(name="yt", bufs=3) as ytp,
            tc.tile_pool(name="ob", bufs=3) as obp,
            tc.tile_pool(name="pp", bufs=2, space="PSUM") as pp,
            tc.tile_pool(name="s2", bufs=2, space="PSUM") as s2p,
            tc.tile_pool(name="op", bufs=2, space="PSUM") as opsp,
        ):
            # --- constants: weights split across DMA queues -----------------
            w_sb = {}
            b_sb = {}
            for name, wdram, bdram in (
                ("q", wq, bq),
                ("k", wk, bk_),
                ("v", wv, None),
            ):
                w_sb[name] = const.tile(
                    [P, XC, CH], BF16, tag=f"w{name}", name=f"w{name}"
                )
                wview = wdram.rearrange("(o p) c -> p o c", p=P)
                for h in range(0, XC, 2):
                    nc.sync.dma_start(
                        w_sb[name][:, h : h + 2, :], wview[:, h : h + 2, :]
                    )
                if bdram is not None:
                    b_sb[name] = const.tile(
                        [CH, 1], F32, tag=f"b{name}", name=f"b{name}"
                    )
                    nc.sync.dma_start(b_sb[name][:], bdram)
            wo_sb = const.tile([CH, D], BF16, tag="wo")
            for h in range(0, D, CW):
                nc.sync.dma_start(wo_sb[:, h : h + CW], wo[:, h : h + CW])
            mask_sb = const.tile([P, pat_w], BF16, tag="mpat")
            nc.sync.dma_start(mask_sb[:], mpat)

            # persistent Q^T/K^T [chan, tok] and V^T-augmented per batch
            qt, kt = {}, {}
            for b in range(B):
                qt[b] = persist.tile([CH, S], BF16, tag=f"qt{b}", name=f"qt{b}")
                kt[b] = persist.tile([CH, S], BF16, tag=f"kt{b}", name=f"kt{b}")
            # vaug[b]: [k-tok, NKB, head, 64 V cols + 64 ones cols]; the ones
            # make the AV matmul emit softmax sums broadcast on rows 64..127
            vaug = {}
            for b in range(B):
                vaug[b] = persist.tile(
                    [P, NKB, HPC, P], BF16, tag=f"vaug{b}", name=f"vaug{b}"
                )
                # contiguous fill; the V columns are overwritten by the V^T
                # evacs, leaving the ones columns that produce the sums rows
                nc.gpsimd.memset(vaug[b][:], 1.0)

            def load_x(b, tg):
                tiles = {}
                g = b * NTG + tg
                for name in ("q", "k", "v"):
                    t = xtp.tile([P, XC, CW], BF16, tag=f"x{name}")
                    for xc in range(XC):
                        nc.sync.dma_start(t[:, xc, :], xdram[name][g, :, xc, :])
                    tiles[name] = t
                return tiles

            def emit_proj(b, name, tg, xtile):
                """Q/K projection of one 512-token group -> qt/kt columns."""
                ps = pp.tile([CH, CW], F32, tag="pp")
                for xc in range(XC):
                    nc.tensor.matmul(
                        ps[:],
                        lhsT=w_sb[name][:, xc, :],
                        rhs=xtile[:, xc, :],
                        start=(xc == 0),
                        stop=(xc == XC - 1),
                    )
                dst = qt if name == "q" else kt
                # evac on the scalar engine (Identity carries the bias; same
                # act table set as Exp, so no table reload)
                nc.scalar.activation(
                    dst[b][:, tg * CW : (tg + 1) * CW],
                    ps[:],
                    mybir.ActivationFunctionType.Identity,
                    bias=b_sb[name][:, 0:1],
                )

            def vt_unit(b, tg, tb, xtile):
                """One 128-token block of V^T: [tok, chan] via swapped matmul."""
                ps = pp.tile([P, HPC, DK], F32, tag="pp")
                for xc in range(XC):
                    nc.tensor.matmul(
                        ps[:],
                        lhsT=xtile[:, xc, tb * P : (tb + 1) * P],
                        rhs=w_sb["v"][:, xc, :],
                        start=(xc == 0),
                        stop=(xc == XC - 1),
                    )
                kb = tg * KPG + tb
                nc.vector.tensor_copy(vaug[b][:, kb, :, 0:DK], ps[:])

            def oproj_piece(tcol, yt, mo):
                op_ps = pp.tile([P, CW], F32, tag="pp")
                nc.tensor.matmul(
                    op_ps[:],
                    lhsT=wo_sb[:, mo * P : (mo + 1) * P],
                    rhs=yt[:],
                    start=True,
                    stop=True,
                )
                ob = obp.tile([P, CW], BF16, tag="ob")
                nc.vector.tensor_copy(ob[:], op_ps[:])
                nc.sync.dma_start(out[mo, tcol], ob[:])

            def attention_col(b, j, tg, vt_units, lazy):
                """One 512-wide q column; drains filler units between blocks."""
                blocks = plan[j]
                q0 = j * CW
                yt = ytp.tile([CH, CW], BF16, tag="yt")
                if not blocks:
                    while vt_units:
                        _, fn = vt_units.popleft()
                        fn()
                    nc.gpsimd.memset(yt[:], 0.0)
                    return yt
                ops = [
                    opsp.tile([P, CW], F32, tag="op", name=f"op{hl}")
                    for hl in range(HPC)
                ]
                nblk = len(blocks)

                def emit_av(i, bk, qa, a2):
                    for hl in range(HPC):
                        nc.tensor.matmul(
                            ops[hl][:, qa:],
                            lhsT=vaug[b][:, bk, hl, :],
                            rhs=a2[:, hl, qa:],
                            start=(i == 0),
                            stop=(i == nblk - 1),
                            skip_group_check=True,
                        )

                pend = None
                debt = 0
                for i, (bk, qa, mixed) in enumerate(blocks):
                    k0 = bk * P
                    s2 = s2p.tile([P, HPC, CW], F32, tag="s2")
                    for hl in range(HPC):
                        hs = slice(hl * DK, (hl + 1) * DK)
                        nc.tensor.matmul(
                            s2[:, hl, qa:],
                            lhsT=kt[b][hs, k0 : k0 + P],
                            rhs=qt[b][hs, q0 + qa : q0 + CW],
                            start=True,
                            stop=True,
                            skip_group_check=True,
                        )
                    a2 = a2p.tile([P, HPC, CW], BF16, tag="a2")
                    nc.scalar.activation(
                        a2[:, :, qa:],
                        s2[:, :, qa:],
                        mybir.ActivationFunctionType.Exp,
                        scale=0.125,
                    )
                    if mixed is not None:
                        off, a_, w_ = mixed
                        nc.vector.tensor_tensor(
                            a2[:, :, a_ : a_ + w_],
                            a2[:, :, a_ : a_ + w_],
                            mask_sb[:, None, off : off + w_].to_broadcast(
                                (P, HPC, w_)
                            ),
                            mybir.AluOpType.mult,
                        )
                    if pend is not None:
                        # this tg's V^T blocks must land before its AVs
                        if pend[1] >= tg * KPG:
                            while vt_units:
                                _, fn = vt_units.popleft()
                                fn()
                        emit_av(*pend)
                    pend = (i, bk, qa, a2)
                    # filler: keep the PE fed while the scalar engine exps
                    debt += 450
                    while debt > 0 and (vt_units or lazy):
                        src = vt_units if vt_units else lazy
                        ns, fn = src.popleft()
                        fn()
                        debt -= ns
                if pend[1] >= tg * KPG:
                    while vt_units:
                        _, fn = vt_units.popleft()
                        fn()
                emit_av(*pend)
                # normalize: rows 64..127 of ops are the sums, pre-broadcast.
                # DVE reads at most one PSUM operand per op, so reciprocal the
                # sums into SBUF, then multiply against the PSUM values.
                for hl in range(HPC):
                    # reciprocal_approx_fast misreads PSUM on HW — bounce the
                    # sums through SBUF first
                    sums = nrm.tile([DK, CW], F32, tag="sums", name=f"sums{hl}")
                    nc.vector.tensor_copy(sums[:], ops[hl][DK:P, :])
                    rec = nrm.tile([DK, CW], F32, tag="rec", name=f"rec{hl}")
                    nc.vector.reciprocal_approx_fast(out=rec[:], in_=sums[:])
                    nc.vector.tensor_tensor(
                        yt[hl * DK : (hl + 1) * DK, :],
                        ops[hl][0:DK, :],
                        rec[:],
                        mybir.AluOpType.mult,
                    )
                return yt

            # --- main schedule ---------------------------------------------
            seq = [(b, tg) for b in range(B) for tg in range(NTG)]
            cur_x = load_x(*seq[0])
            lazy = deque()
            for si, (b, tg) in enumerate(seq):
                nxt_x = load_x(*seq[si + 1]) if si + 1 < len(seq) else None
                emit_proj(b, "q", tg, cur_x["q"])
                emit_proj(b, "k", tg, cur_x["k"])
                vt_units = deque(
                    (900, partial(vt_unit, b, tg, tb, cur_x["v"]))
                    for tb in range(KPG)
                )
                yt = attention_col(b, tg, tg, vt_units, lazy)
                while vt_units:
                    _, fn = vt_units.popleft()
                    fn()
                tcol = b * NJ + tg
                lazy.extend(
                    (450, partial(oproj_piece, tcol, yt, mo)) for mo in range(MO)
                )
                cur_x = nxt_x
            while lazy:
                _, fn = lazy.popleft()
                fn()
    nc.compile()
    return nc


def _get_module(plan, pat_w):
    key = (plan, pat_w)
    if key not in _BUILD_CACHE:
        _BUILD_CACHE[key] = _build(plan, pat_w)
    return _BUILD_CACHE[key]


def _prep_inputs(query, key, value, mask, W_q, b_q, W_k, b_k, W_v, b_v, W_o, b_o):
    def xt_of(x):
        x2 = np.asarray(x, np.float32).reshape(TOK, D)
        xt = x2.T.astype(NPBF16)  # (D, TOK)
        xt = xt.reshape(XC, P, B * NTG, CW).transpose(2, 1, 0, 3)
        return np.ascontiguousarray(xt)  # (NTT, P, XC, CW)

    xq, xk, xv = xt_of(query), xt_of(key), xt_of(value)
    plan, pat_arr = _analyze_mask(mask)
    mpat = np.ascontiguousarray(pat_arr).astype(NPBF16)

    W_q = np.asarray(W_q, np.float32)
    W_k = np.asarray(W_k, np.float32)
    W_v = np.asarray(W_v, np.float32)
    W_o = np.asarray(W_o, np.float32)

    in_maps = []
    for c in range(N_CORES):
        cs = slice(c * CH, (c + 1) * CH)
        in_maps.append(
            {
                "xq": xq,
                "xk": xk,
                "xv": xv,
                "wq": np.ascontiguousarray(W_q[cs, :].T).astype(NPBF16),
                "wk": np.ascontiguousarray(W_k[cs, :].T).astype(NPBF16),
                "wv": np.ascontiguousarray(W_v[cs, :].T).astype(NPBF16),
                "wo": np.ascontiguousarray(W_o[:, cs].T).astype(NPBF16),
                "bq": np.asarray(b_q, np.float32)[cs].reshape(CH, 1).copy(),
                "bk": np.asarray(b_k, np.float32)[cs].reshape(CH, 1).copy(),
                "mpat": mpat,
            }
        )
    return plan, mpat.shape[1], in_maps


def run(inputs, trace=False, trace_cores=None):
    """Build (cached), run on 8 cores, return (final_output, BassKernelResults)."""
    plan, pat_w, in_maps = _prep_inputs(**inputs)
    nc = _get_module(plan, pat_w)
    res = bass_utils.run_bass_kernel_spmd(
        nc,
        in_maps,
        core_ids=list(range(N_CORES)),
        trace=trace,
        trace_cores=trace_cores,
    )
    acc = np.zeros((MO, B * NJ, P, CW), np.float32)
    for c in range(N_CORES):
        acc += res.results[c]["out"].astype(np.float32)
    acc = acc.transpose(0, 2, 1, 3).reshape(D, TOK)
    # v-bias contributes W_o @ b_v to every token; fold it into the out bias
    bo_eff = np.asarray(inputs["b_o"], np.float32) + np.asarray(
        inputs["W_o"], np.float32
    ) @ np.asarray(inputs["b_v"], np.float32)
    final = acc.T + bo_eff[None, :]
    return final.reshape(B, S, D), res


def kernel(**inputs):
    return run(inputs, trace=False)[0]
